# revision 1
# baseline (speedup 1.0000x reference)
"""MemoryCrossAttention Trainium2 Bass kernel.

8-core data-parallel over query rows: core c handles batch c//2, row-half
c%2 (2048 rows). K/V come from the 256 memory tokens, computed per core.
All matmuls run as float32r (full PE rate at N>=512, ~1e-4 rel precision).
RMSNorm is computed on-chip via a squares pass + ones-matmul partition
reduction; softmax mask folds into the exp bias (per-partition, scoresT
layout); the softmax denominator is a ones-matmul over probsT with the
reciprocal fused into the attention-output eviction.
"""
from concourse.bass_utils import run_bass_kernel_spmd


from contextlib import ExitStack

import concourse.bass as bass
import concourse.tile as tile
from concourse import mybir

F32 = mybir.dt.float32
F32R = mybir.dt.float32r
BF16 = mybir.dt.bfloat16
P = 128


def build(nc, H, NH, R, M, eps=1e-6, phases=4):
    HD = 128
    assert H == NH * HD
    KT = H // P           # contraction tiles
    LQ = R // 512         # 512-wide l chunks
    NHTP = NH // 2        # head pairs (Q/G/O weight streaming)
    MT = M // P           # memory-token partition tiles (2)
    KH = min(8, NH)       # heads per K-proj psum group
    NKG = NH // KH        # K-proj head groups
    KGW = KH * P          # K-proj weight tile width
    NVC = max(1, (NH * HD) // 512)  # V d-chunks of 512
    scale = HD ** -0.5

    xT = nc.dram_tensor("xT", [H, R], F32R, kind="ExternalInput")
    memT = nc.dram_tensor("memT", [H, M], F32R, kind="ExternalInput")
    maskb = nc.dram_tensor("maskb", [P, MT], F32, kind="ExternalInput")
    wqT = nc.dram_tensor("wqT", [NHTP, KT, P, 256], F32R, kind="ExternalInput")
    wgT = nc.dram_tensor("wgT", [NHTP, KT, P, 256], F32R, kind="ExternalInput")
    woT = nc.dram_tensor("woT", [NHTP, KT, P, 256], F32R, kind="ExternalInput")
    wkT = nc.dram_tensor("wkT", [NKG, KT, P, KGW], F32R, kind="ExternalInput")
    wvT = nc.dram_tensor("wvT", [NVC, KT, P, 512], F32R, kind="ExternalInput")
    outT = nc.dram_tensor("outT", [H, R], F32, kind="ExternalOutput")

    with tile.TileContext(nc) as tc, ExitStack() as ctx:
        dram = ctx.enter_context(tc.tile_pool(name="dram", bufs=1, space="DRAM"))
        qspill = dram.tile([H, R], F32R)
        gspill = dram.tile([H, R], F32)
        aspill = dram.tile([H, R], F32R)
        s_scr = dram.tile([R], F32)
        rd_scr = dram.tile([NH, R], F32)

        const = ctx.enter_context(tc.tile_pool(name="const", bufs=1))
        ones_f32 = const.tile([P, 1], F32)
        nc.vector.memset(ones_f32, 1.0)
        ones_sb = const.tile([P, 1], F32R)
        nc.vector.tensor_copy(ones_sb, ones_f32)
        eps_sb = const.tile([1, 1], F32)
        nc.vector.memset(eps_sb, eps)
        mask_sb = const.tile([P, MT], F32)
        nc.sync.dma_start(out=mask_sb, in_=maskb[:])

        # persistent: K/V stay for phases B-C
        kvpool = ctx.enter_context(tc.tile_pool(name="kv", bufs=1))
        kT_big = kvpool.tile([P, NH, M], F32R)    # [d, h, m]
        vmd_big = kvpool.tile([P, MT, H], F32R)   # [m, mt, d_full]

        with tc.tile_pool(name="x", bufs=1) as xpool:
            x_big = xpool.tile([P, KT, R], F32R)  # xT tiles; becomes xnT in place

            # ================= Phase A: load x, rmsnorm =================
            for kt in range(KT):
                nc.sync.dma_start(out=x_big[:, kt, :],
                                  in_=xT[kt * P:(kt + 1) * P, :])

            with tc.tile_pool(name="x2", bufs=2) as x2p, \
                 tc.tile_pool(name="ssqp", bufs=1, space="PSUM") as ssqp, \
                 tc.tile_pool(name="sp", bufs=1) as sp:
                ssq = [ssqp.tile([1, 512], F32, name=f"ssq{i}") for i in range(LQ)]
                for kt in range(KT):
                    x2 = x2p.tile([P, R], F32R)
                    nc.vector.tensor_mul(x2, x_big[:, kt, :], x_big[:, kt, :])
                    for lq in range(LQ):
                        nc.tensor.matmul(
                            ssq[lq], ones_sb, x2[:, lq * 512:(lq + 1) * 512],
                            start=(kt == 0), stop=(kt == KT - 1))
                s_sb = sp.tile([1, R], F32)
                rs_sb = sp.tile([1, R], F32)
                for lq in range(LQ):
                    nc.scalar.activation(
                        s_sb[:, lq * 512:(lq + 1) * 512], ssq[lq],
                        mybir.ActivationFunctionType.Sqrt,
                        bias=eps_sb, scale=1.0 / H)
                nc.vector.reciprocal(rs_sb, s_sb)
                nc.sync.dma_start(out=s_scr[:], in_=rs_sb[0:1, :])

            with tc.tile_pool(name="sbc", bufs=1) as sbcp:
                s_bc = sbcp.tile([P, R], F32)
                s_ap = s_scr[:]
                nc.sync.dma_start(
                    out=s_bc,
                    in_=bass.AP(tensor=s_ap.tensor, offset=s_ap.offset,
                                ap=[[0, P]] + s_ap.ap))
                for kt in range(KT):
                    nc.vector.tensor_mul(x_big[:, kt, :], x_big[:, kt, :], s_bc)

            # ============= Phase B1: K/V from memory tokens =============
            with tc.tile_pool(name="memp", bufs=1) as memp:
                mem_big = memp.tile([P, KT, M], F32R)
                for kt in range(KT):
                    nc.sync.dma_start(out=mem_big[:, kt, :],
                                      in_=memT[kt * P:(kt + 1) * P, :])

                with tc.tile_pool(name="wkst", bufs=3) as wkst, \
                     tc.tile_pool(name="kps", bufs=1, space="PSUM") as kps:
                    for kg in range(NKG):
                        kpsum = [kps.tile([P, M], F32, name=f"kpsum{i}")
                                 for i in range(KH)]
                        for kt in range(KT):
                            wk_t = wkst.tile([P, KGW], F32R)
                            nc.sync.dma_start(out=wk_t, in_=wkT[kg, kt])
                            for hh in range(KH):
                                nc.tensor.matmul(
                                    kpsum[hh], wk_t[:, hh * P:(hh + 1) * P],
                                    mem_big[:, kt, :],
                                    start=(kt == 0), stop=(kt == KT - 1))
                        for hh in range(KH):
                            nc.vector.tensor_copy(kT_big[:, kg * KH + hh, :],
                                                  kpsum[hh])

                with tc.tile_pool(name="wvst", bufs=3) as wvst, \
                     tc.tile_pool(name="vps", bufs=1, space="PSUM") as vps:
                    for dc in range(NVC):
                        vpsum = [vps.tile([P, 512], F32, name=f"vpsum{i}")
                                 for i in range(MT)]
                        for kt in range(KT):
                            wv_t = wvst.tile([P, 512], F32R)
                            nc.sync.dma_start(out=wv_t, in_=wvT[dc, kt])
                            for mt in range(MT):
                                nc.tensor.matmul(
                                    vpsum[mt],
                                    mem_big[:, kt, mt * P:(mt + 1) * P],
                                    wv_t,
                                    start=(kt == 0), stop=(kt == KT - 1))
                        for mt in range(MT):
                            nc.vector.tensor_copy(
                                vmd_big[:, mt, dc * 512:(dc + 1) * 512], vpsum[mt])

            # ============= Phase B2: Q and gate projections =============
            if phases < 2:
                return
            with tc.tile_pool(name="wqst", bufs=3) as wqst, \
                 tc.tile_pool(name="qps", bufs=1, space="PSUM") as qps, \
                 tc.tile_pool(name="qbuf", bufs=4) as qbufp, \
                 tc.tile_pool(name="gbuf", bufs=4) as gbufp:
                for htp in range(NHTP):
                    qpsum = [[qps.tile([P, 512], F32, name=f"qpsum{i}_{j}")
                              for j in range(LQ)] for i in range(2)]
                    for kt in range(KT):
                        wq_t = wqst.tile([P, 256], F32R)
                        nc.sync.dma_start(out=wq_t, in_=wqT[htp, kt])
                        for h2 in range(2):
                            for lq in range(LQ):
                                nc.tensor.matmul(
                                    qpsum[h2][lq], wq_t[:, h2 * P:(h2 + 1) * P],
                                    x_big[:, kt, lq * 512:(lq + 1) * 512],
                                    start=(kt == 0), stop=(kt == KT - 1))
                    for h2 in range(2):
                        ho = htp * 2 + h2
                        for lq in range(LQ):
                            qb = qbufp.tile([P, 512], F32R, name="qb")
                            nc.scalar.copy(qb, qpsum[h2][lq])
                            nc.sync.dma_start(
                                out=qspill[ho * P:(ho + 1) * P,
                                           lq * 512:(lq + 1) * 512],
                                in_=qb)

                for htp in range(NHTP):
                    gpsum = [[qps.tile([P, 512], F32, name=f"qpsum{i}_{j}")
                              for j in range(LQ)] for i in range(2)]
                    for kt in range(KT):
                        wg_t = wqst.tile([P, 256], F32R)
                        nc.sync.dma_start(out=wg_t, in_=wgT[htp, kt])
                        for h2 in range(2):
                            for lq in range(LQ):
                                nc.tensor.matmul(
                                    gpsum[h2][lq], wg_t[:, h2 * P:(h2 + 1) * P],
                                    x_big[:, kt, lq * 512:(lq + 1) * 512],
                                    start=(kt == 0), stop=(kt == KT - 1))
                    for h2 in range(2):
                        ho = htp * 2 + h2
                        for lq in range(LQ):
                            gb = gbufp.tile([P, 512], F32, name="gb")
                            nc.scalar.activation(
                                gb, gpsum[h2][lq],
                                mybir.ActivationFunctionType.Sigmoid)
                            nc.sync.dma_start(
                                out=gspill[ho * P:(ho + 1) * P,
                                           lq * 512:(lq + 1) * 512],
                                in_=gb)

        # ================= Phase C: attention per head =================
        if phases < 3:
            return
        with tc.tile_pool(name="qh", bufs=2) as qhp, \
             tc.tile_pool(name="probs", bufs=2) as probsp, \
             tc.tile_pool(name="rden", bufs=2) as rdenp, \
             tc.tile_pool(name="asb", bufs=2) as asbp, \
             tc.tile_pool(name="sps", bufs=4, space="PSUM") as sps, \
             tc.tile_pool(name="dps", bufs=2, space="PSUM") as dps, \
             tc.tile_pool(name="aps", bufs=2, space="PSUM") as aps:
            for h in range(NH):
                qh = qhp.tile([P, R], F32R, name="qh")
                nc.sync.dma_start(out=qh, in_=qspill[h * P:(h + 1) * P, :])

                probs = probsp.tile([P, MT, R], F32R, name="probs")
                for mt in range(MT):
                    for lq in range(LQ):
                        spsum = sps.tile([P, 512], F32, name="spsum")
                        nc.tensor.matmul(
                            spsum, kT_big[:, h, mt * P:(mt + 1) * P],
                            qh[:, lq * 512:(lq + 1) * 512],
                            start=True, stop=True)
                        nc.scalar.activation(
                            probs[:, mt, lq * 512:(lq + 1) * 512], spsum,
                            mybir.ActivationFunctionType.Exp,
                            bias=mask_sb[:, mt:mt + 1], scale=scale)

                rden = rdenp.tile([1, R], F32, name="rden")
                for lq in range(LQ):
                    dpsum = dps.tile([1, 512], F32, name="dpsum")
                    for mt in range(MT):
                        nc.tensor.matmul(
                            dpsum, ones_sb,
                            probs[:, mt, lq * 512:(lq + 1) * 512],
                            start=(mt == 0), stop=(mt == MT - 1))
                    nc.vector.reciprocal(rden[:, lq * 512:(lq + 1) * 512], dpsum)
                nc.sync.dma_start(out=rd_scr[h:h + 1, :], in_=rden[0:1, :])

                rden_bc = rdenp.tile([P, R], F32, name="rden_bc")
                rd_ap = rd_scr[h, :]
                nc.sync.dma_start(
                    out=rden_bc,
                    in_=bass.AP(tensor=rd_ap.tensor, offset=rd_ap.offset,
                                ap=[[0, P]] + rd_ap.ap))

                attn_sb = asbp.tile([P, R], F32R, name="attn_sb")
                for lq in range(LQ):
                    apsum = aps.tile([P, 512], F32, name="apsum")
                    for mt in range(MT):
                        nc.tensor.matmul(
                            apsum, vmd_big[:, mt, h * P:(h + 1) * P],
                            probs[:, mt, lq * 512:(lq + 1) * 512],
                            start=(mt == 0), stop=(mt == MT - 1))
                    nc.vector.tensor_mul(
                        attn_sb[:, lq * 512:(lq + 1) * 512], apsum,
                        rden_bc[:, lq * 512:(lq + 1) * 512])
                nc.sync.dma_start(out=aspill[h * P:(h + 1) * P, :], in_=attn_sb)

        # ================= Phase D: O-proj + gate =================
        if phases < 4:
            return
        with tc.tile_pool(name="at", bufs=1) as atp, \
             tc.tile_pool(name="wost", bufs=3) as wost, \
             tc.tile_pool(name="gin", bufs=2) as ginp, \
             tc.tile_pool(name="osb", bufs=2) as osbp, \
             tc.tile_pool(name="ops", bufs=1, space="PSUM") as ops:
            at_big = atp.tile([P, KT, R], F32R)
            for kt in range(KT):
                nc.sync.dma_start(out=at_big[:, kt, :],
                                  in_=aspill[kt * P:(kt + 1) * P, :])
            for htp in range(NHTP):
                opsum = [[ops.tile([P, 512], F32, name=f"opsum{i}_{j}")
                          for j in range(LQ)] for i in range(2)]
                for kt in range(KT):
                    wo_t = wost.tile([P, 256], F32R)
                    nc.sync.dma_start(out=wo_t, in_=woT[htp, kt])
                    for h2 in range(2):
                        for lq in range(LQ):
                            nc.tensor.matmul(
                                opsum[h2][lq], wo_t[:, h2 * P:(h2 + 1) * P],
                                at_big[:, kt, lq * 512:(lq + 1) * 512],
                                start=(kt == 0), stop=(kt == KT - 1))
                for h2 in range(2):
                    ho = htp * 2 + h2
                    g_in = ginp.tile([P, R], F32, name="g_in")
                    nc.sync.dma_start(out=g_in,
                                      in_=gspill[ho * P:(ho + 1) * P, :])
                    o_sb = osbp.tile([P, R], F32, name="o_sb")
                    for lq in range(LQ):
                        nc.vector.tensor_mul(
                            o_sb[:, lq * 512:(lq + 1) * 512], opsum[h2][lq],
                            g_in[:, lq * 512:(lq + 1) * 512])
                    nc.sync.dma_start(out=outT[ho * P:(ho + 1) * P, :], in_=o_sb)

    nc.compile()
    return nc


def prep_inputs(hs_slice, mem_b, mask_b, norm_w, wq, wk, wv, wo, wg, NH):
    """Host-side prep for one core. hs_slice [R, H], mem_b [M, H], mask_b [M]."""
    import numpy as np
    H = hs_slice.shape[1]
    M = mem_b.shape[0]
    P = 128
    KT = H // P
    KH = min(8, NH)
    KGW = KH * P

    def tile_w(wT, width):
        # wT [H, H] -> [H//width, KT, 128, width]
        n = wT.shape[1] // width
        return np.ascontiguousarray(
            wT.reshape(KT, P, n, width).transpose(2, 0, 1, 3))

    wq_n = (wq * norm_w[None, :]).T.astype(np.float32)   # [in, out]
    wg_n = (wg * norm_w[None, :]).T.astype(np.float32)
    wo_t = wo.T.astype(np.float32)
    wk_t = wk.T.astype(np.float32)
    wv_t = wv.T.astype(np.float32)

    maskb = np.where(mask_b, 0.0, -50.0).astype(np.float32)
    maskb = np.ascontiguousarray(maskb.reshape(M // P, P).T)  # [128, MT]

    return {
        "xT": np.ascontiguousarray(hs_slice.T.astype(np.float32)),
        "memT": np.ascontiguousarray(mem_b.T.astype(np.float32)),
        "maskb": maskb,
        "wqT": tile_w(wq_n, 256),
        "wgT": tile_w(wg_n, 256),
        "woT": tile_w(wo_t, 256),
        "wkT": tile_w(wk_t, KGW),
        "wvT": tile_w(wv_t, 512),
    }


import numpy as np

_H, _NH, _HD, _M = 2048, 16, 128, 256
_B, _L = 4, 4096
_RPC = 2048          # rows per core
_NCORES = 8
_EPS = 1e-6

_nc_cache = [None]


def _prep_core(hs_slice, mem_b, mask_b, shared):
    inp = dict(shared)
    inp["xT"] = np.ascontiguousarray(hs_slice.T)
    inp["memT"] = np.ascontiguousarray(mem_b.T)
    maskb = np.where(mask_b, 0.0, -50.0).astype(np.float32)
    inp["maskb"] = np.ascontiguousarray(maskb.reshape(_M // 128, 128).T)
    return inp


def _tile_w(wT, width):
    KT = wT.shape[0] // 128
    n = wT.shape[1] // width
    return np.ascontiguousarray(
        wT.reshape(KT, 128, n, width).transpose(2, 0, 1, 3))


def kernel(hidden_states, memory_tokens, memory_mask, norm_w,
           wq, wk, wv, wo, wg):
    import concourse.bacc as bacc

    hs = np.asarray(hidden_states, dtype=np.float32)
    mem = np.asarray(memory_tokens, dtype=np.float32)
    mask = np.asarray(memory_mask)
    norm_w = np.asarray(norm_w, dtype=np.float32)

    wq_n = (np.asarray(wq, dtype=np.float32) * norm_w[None, :]).T
    wg_n = (np.asarray(wg, dtype=np.float32) * norm_w[None, :]).T
    shared = {
        "wqT": _tile_w(np.ascontiguousarray(wq_n), 256),
        "wgT": _tile_w(np.ascontiguousarray(wg_n), 256),
        "woT": _tile_w(np.ascontiguousarray(np.asarray(wo, dtype=np.float32).T), 256),
        "wkT": _tile_w(np.ascontiguousarray(np.asarray(wk, dtype=np.float32).T), 1024),
        "wvT": _tile_w(np.ascontiguousarray(np.asarray(wv, dtype=np.float32).T), 512),
    }

    in_maps = []
    for c in range(_NCORES):
        b, half = c // 2, c % 2
        hs_slice = hs[b, half * _RPC:(half + 1) * _RPC, :]
        in_maps.append(_prep_core(hs_slice, mem[b], mask[b], shared))

    if _nc_cache[0] is None:
        nc = bacc.Bacc(None, target_bir_lowering=False, debug=False)
        build(nc, _H, _NH, _RPC, _M, eps=_EPS)
        _nc_cache[0] = nc
    nc = _nc_cache[0]

    import os
    trace = os.environ.get("KERNEL_TRACE") == "1"
    res = run_bass_kernel_spmd(nc, in_maps, core_ids=list(range(_NCORES)),
                               trace=trace)
    kernel.last_result = res

    out = np.empty((_B, _L, _H), dtype=np.float32)
    for c in range(_NCORES):
        b, half = c // 2, c % 2
        out[b, half * _RPC:(half + 1) * _RPC, :] = res.results[c]["outT"].T
    return out



# revision 5
# speedup vs baseline: 1.4590x; 1.4590x over previous
"""MemoryCrossAttention Trainium2 Bass kernel (v2, bf16 pipeline).

8-core data-parallel over query rows: core c handles batch c//2, row-half
c%2 (R=2048 rows). All matmuls run in bf16 (FWL weight loads, f32 PSUM
accumulation). Attention for head pair p is interleaved between projection
units (Q htp / G htp) so the PE stays dense and HAM-warm. PSUM budget:
4 banks proj (double-buffered groups) + 2 scores + 1 denom + 1 attn.
Softmax denominators use reciprocal_approx_fast + a DRAM stride-0
broadcast; attention output is normalized post-eviction on DVE.
"""
from contextlib import ExitStack

import numpy as np

import concourse.bass as bass
import concourse.tile as tile
from concourse import mybir
from concourse.bass_utils import run_bass_kernel_spmd

F32 = mybir.dt.float32
BF16 = mybir.dt.bfloat16
P = 128

_H, _NH, _HD, _M = 2048, 16, 128, 256
_B, _L = 4, 4096
_R = 2048            # rows per core
_NCORES = 8
_EPS = 1e-6
_KT = _H // P        # 16 contraction tiles
_MT = _M // P        # 2
_NHTP = _NH // 2     # 8 head pairs
_SCALE = _HD ** -0.5


def _bcast_ap(ap, p=P):
    return bass.AP(tensor=ap.tensor, offset=ap.offset, ap=[[0, p]] + ap.ap)


def build(nc):
    H, NH, R, M, KT, MT, NHTP = _H, _NH, _R, _M, _KT, _MT, _NHTP
    LH = R // 1024       # 2 row-halves (1024) per R
    LQ = R // 512        # 4 512-chunks

    xT = nc.dram_tensor("xT", [H, R], F32, kind="ExternalInput")
    memTb = nc.dram_tensor("memTb", [P, KT * M], BF16, kind="ExternalInput")
    maskb = nc.dram_tensor("maskb", [P, MT], F32, kind="ExternalInput")
    wqTb = nc.dram_tensor("wqTb", [NHTP, P, KT * 256], BF16, kind="ExternalInput")
    wgTb = nc.dram_tensor("wgTb", [NHTP, P, KT * 256], BF16, kind="ExternalInput")
    woTb = nc.dram_tensor("woTb", [NHTP, P, KT * 256], BF16, kind="ExternalInput")
    wkTb = nc.dram_tensor("wkTb", [4, P, KT * 512], BF16, kind="ExternalInput")
    wvTb = nc.dram_tensor("wvTb", [4, P, KT * 512], BF16, kind="ExternalInput")
    outT = nc.dram_tensor("outT", [H, R], F32, kind="ExternalOutput")

    with tile.TileContext(nc) as tc, ExitStack() as ctx:
        dram = ctx.enter_context(tc.tile_pool(name="dram", bufs=1, space="DRAM"))
        aspill = dram.tile([NH, P, R], BF16)
        gspill = dram.tile([NH, P, R], BF16)
        rs_scr = dram.tile([R], F32)
        rd_scr = dram.tile([NH, R], BF16)

        const = ctx.enter_context(tc.tile_pool(name="const", bufs=1))
        ones_f32 = const.tile([P, 1], F32)
        nc.vector.memset(ones_f32, 1.0)
        ones_bf = const.tile([P, 1], BF16)
        nc.vector.tensor_copy(ones_bf, ones_f32)
        eps_sb = const.tile([1, 1], F32)
        nc.vector.memset(eps_sb, _EPS)
        mask_sb = const.tile([P, MT], F32)
        nc.sync.dma_start(out=mask_sb, in_=maskb[:])

        # persistent SBUF tensors
        kv = ctx.enter_context(tc.tile_pool(name="kv", bufs=1))
        mem_sb = kv.tile([P, KT, M], BF16)
        nc.sync.dma_start(out=mem_sb, in_=memTb[:])
        kT_sb = kv.tile([P, NH, M], BF16)       # [d, h, m]
        vmd_sb = kv.tile([P, MT, H], BF16)      # [m, mt, h*d]
        sbc_p = ctx.enter_context(tc.tile_pool(name="sbc", bufs=1))
        s_bc = sbc_p.tile([P, R], F32)

        # ============ Phase A: x load/cast/ssq + K/V projections ============
        with tc.tile_pool(name="xbfp", bufs=1) as xbfp:
            xbf = xbfp.tile([P, KT, R], BF16)

            with tc.tile_pool(name="xf", bufs=3) as xfp, \
                 tc.tile_pool(name="x2", bufs=2) as x2p:
                x2s = []
                for kt in range(KT):
                    xf = xfp.tile([P, R], F32, name="xf")
                    nc.sync.dma_start(out=xf, in_=xT[kt * P:(kt + 1) * P, :])
                    nc.gpsimd.tensor_copy(xbf[:, kt, :], xf)
                    x2 = x2p.tile([P, R], BF16, name="x2")
                    nc.vector.tensor_mul(x2, xf, xf)
                    x2s.append(x2)

                # K proj: 4 rounds x 4 heads; psum [128, 4, 512] (4 banks)
                with tc.tile_pool(name="wkst", bufs=2) as wkst, \
                     tc.tile_pool(name="kps", bufs=2, space="PSUM") as kps:
                    for rnd in range(4):
                        wk_t = wkst.tile([P, KT * 512], BF16, name="wk")
                        nc.sync.dma_start(out=wk_t, in_=wkTb[rnd])
                        kpsum = kps.tile([P, 2048], F32, name="kpsum")
                        for kt in range(KT):
                            for hh in range(4):
                                nc.tensor.matmul(
                                    kpsum[:, hh * 512:hh * 512 + M],
                                    wk_t[:, kt * 512 + hh * P:
                                         kt * 512 + (hh + 1) * P],
                                    mem_sb[:, kt, :],
                                    start=(kt == 0), stop=(kt == KT - 1))
                        for hh in range(4):
                            nc.vector.tensor_copy(
                                kT_sb[:, rnd * 4 + hh, :],
                                kpsum[:, hh * 512:hh * 512 + M])

                # V proj: 4 dc rounds; psum [128, 2, 512] (2 banks) x2 bufs
                with tc.tile_pool(name="wvst", bufs=2) as wvst, \
                     tc.tile_pool(name="vps", bufs=2, space="PSUM") as vps:
                    for dc in range(4):
                        wv_t = wvst.tile([P, KT * 512], BF16, name="wv")
                        nc.sync.dma_start(out=wv_t, in_=wvTb[dc])
                        vpsum = vps.tile([P, 1024], F32, name="vpsum")
                        for kt in range(KT):
                            for mt in range(MT):
                                nc.tensor.matmul(
                                    vpsum[:, mt * 512:(mt + 1) * 512],
                                    mem_sb[:, kt, mt * P:(mt + 1) * P],
                                    wv_t[:, kt * 512:(kt + 1) * 512],
                                    start=(kt == 0), stop=(kt == KT - 1))
                        for mt in range(MT):
                            nc.vector.tensor_copy(
                                vmd_sb[:, mt, dc * 512:(dc + 1) * 512],
                                vpsum[:, mt * 512:(mt + 1) * 512])

                # ssq partition-sums (after K/V in program order; PE catches up)
                with tc.tile_pool(name="ssqp", bufs=1, space="PSUM") as ssqp, \
                     tc.tile_pool(name="sp", bufs=1) as sp:
                    ssq = ssqp.tile([1, R], F32)
                    for kt in range(KT):
                        for lq in range(LQ):
                            nc.tensor.matmul(
                                ssq[0:1, lq * 512:(lq + 1) * 512], ones_bf,
                                x2s[kt][:, lq * 512:(lq + 1) * 512],
                                start=(kt == 0), stop=(kt == KT - 1))
                    s_sb = sp.tile([1, R], F32)
                    rs_sb = sp.tile([1, R], F32)
                    nc.scalar.activation(
                        s_sb, ssq[0:1, :], mybir.ActivationFunctionType.Sqrt,
                        bias=eps_sb, scale=1.0 / H)
                    nc.vector.reciprocal_approx_fast(rs_sb, s_sb)
                    nc.sync.dma_start(out=rs_scr[:], in_=rs_sb[0:1, :])

            nc.sync.dma_start(out=s_bc, in_=_bcast_ap(rs_scr[:]))
            for kt in range(KT):
                nc.vector.tensor_mul(xbf[:, kt, :], xbf[:, kt, :], s_bc)

            # ============ Main interleaved units ============
            # unit u: proj (Q htp u for u<8, G htp u-8 for u>=8)
            # around unit u: attention stages for head pair u-1 / u-2
            with tc.tile_pool(name="wst", bufs=2) as wst, \
                 tc.tile_pool(name="pps", bufs=2, space="PSUM") as pps, \
                 tc.tile_pool(name="qp", bufs=2) as qp, \
                 tc.tile_pool(name="gstg", bufs=3) as gstg, \
                 tc.tile_pool(name="sps", bufs=1, space="PSUM") as sps, \
                 tc.tile_pool(name="dps", bufs=1, space="PSUM") as dps, \
                 tc.tile_pool(name="aps", bufs=1, space="PSUM") as aps, \
                 tc.tile_pool(name="probs", bufs=3) as probsp, \
                 tc.tile_pool(name="aup", bufs=4) as aup, \
                 tc.tile_pool(name="asb", bufs=2) as asbp, \
                 tc.tile_pool(name="rdp", bufs=2) as rdp, \
                 tc.tile_pool(name="rbc", bufs=2) as rbcp:

                q_tiles = {}      # pair -> tile [P, 2, R]
                probs_t = {}      # head -> tile [P, MT, R]
                au_t = {}         # head -> tile [P, R] bf16 (unnormalized)
                rbc_t = {}        # head -> tile [P, R] bf16 (1/den bcast)

                def emit_scores_exp(pair):
                    qt = q_tiles[pair]
                    for hh in range(2):
                        h = pair * 2 + hh
                        pr = probsp.tile([P, MT, R], BF16, name="probs")
                        probs_t[h] = pr
                        for mt in range(MT):
                            for lh in range(LH):
                                spsum = sps.tile([P, 1024], F32, name="spsum")
                                for j in range(2):
                                    sl = slice((lh * 2 + j) * 512,
                                               (lh * 2 + j + 1) * 512)
                                    nc.tensor.matmul(
                                        spsum[:, j * 512:(j + 1) * 512],
                                        kT_sb[:, h, mt * P:(mt + 1) * P],
                                        qt[:, hh, sl], start=True, stop=True)
                                nc.scalar.activation(
                                    pr[:, mt, lh * 1024:(lh + 1) * 1024],
                                    spsum, mybir.ActivationFunctionType.Exp,
                                    bias=mask_sb[:, mt:mt + 1], scale=_SCALE)

                def emit_denom_attn(pair):
                    for hh in range(2):
                        h = pair * 2 + hh
                        pr = probs_t[h]
                        rden = rdp.tile([1, R], F32, name="rden")
                        for lq in range(LQ):
                            sl = slice(lq * 512, (lq + 1) * 512)
                            dpsum = dps.tile([1, 512], F32, name="dpsum")
                            for mt in range(MT):
                                nc.tensor.matmul(
                                    dpsum, ones_bf, pr[:, mt, sl],
                                    start=(mt == 0), stop=(mt == MT - 1))
                            nc.vector.reciprocal_approx_fast(rden[:, sl], dpsum)
                        nc.gpsimd.dma_start(out=rd_scr[h:h + 1, :],
                                            in_=rden[0:1, :])
                        rbc = rbcp.tile([P, R], BF16, name="rbc")
                        rbc_t[h] = rbc
                        nc.sync.dma_start(out=rbc, in_=_bcast_ap(rd_scr[h, :]))
                        au = aup.tile([P, R], BF16, name="au")
                        au_t[h] = au
                        for lq in range(LQ):
                            sl = slice(lq * 512, (lq + 1) * 512)
                            apsum = aps.tile([P, 512], F32, name="apsum")
                            for mt in range(MT):
                                nc.tensor.matmul(
                                    apsum, vmd_sb[:, mt, h * P:(h + 1) * P],
                                    pr[:, mt, sl],
                                    start=(mt == 0), stop=(mt == MT - 1))
                            nc.scalar.copy(au[:, sl], apsum)
                        del probs_t[h]

                def emit_norm_spill(pair):
                    for hh in range(2):
                        h = pair * 2 + hh
                        attn_sb = asbp.tile([P, R], BF16, name="attn_sb")
                        nc.vector.tensor_mul(attn_sb, au_t[h], rbc_t[h])
                        nc.sync.dma_start(out=aspill[h], in_=attn_sb)
                        del au_t[h], rbc_t[h]

                def emit_proj(u):
                    wt = wst.tile([P, KT * 256], BF16, name="wt")
                    src = wqTb if u < 8 else wgTb
                    htp = u % 8
                    nc.sync.dma_start(out=wt, in_=src[htp])
                    if u < 8:
                        qt = qp.tile([P, 2, R], BF16, name="qt")
                        q_tiles[htp] = qt
                    for h2 in range(2):
                        for rh in range(LH):
                            ppsum = pps.tile([P, 1024], F32, name="ppsum")
                            for kt in range(KT):
                                for lq in range(2):
                                    sl = slice(rh * 1024 + lq * 512,
                                               rh * 1024 + (lq + 1) * 512)
                                    nc.tensor.matmul(
                                        ppsum[:, lq * 512:(lq + 1) * 512],
                                        wt[:, kt * 256 + h2 * P:
                                           kt * 256 + (h2 + 1) * P],
                                        xbf[:, kt, sl],
                                        start=(kt == 0), stop=(kt == KT - 1))
                            osl = slice(rh * 1024, (rh + 1) * 1024)
                            if u < 8:
                                nc.scalar.copy(
                                    q_tiles[htp][:, h2, osl], ppsum)
                            else:
                                g = gstg.tile([P, 1024], BF16, name="g")
                                nc.scalar.activation(
                                    g, ppsum,
                                    mybir.ActivationFunctionType.Sigmoid)
                                nc.sync.dma_start(
                                    out=gspill[htp * 2 + h2][:, osl], in_=g)

                for u in range(16):
                    pair = u - 1
                    if 0 <= pair < NHTP:
                        emit_scores_exp(pair)
                    emit_proj(u)
                    if 0 <= pair < NHTP:
                        emit_denom_attn(pair)
                        if pair >= 1:
                            emit_norm_spill(pair - 1)
                        if pair == NHTP - 1:
                            emit_norm_spill(pair)
                        else:
                            pass
                    if u < 8:
                        pass

        # ============ Phase D: O proj + gate ============
        with tc.tile_pool(name="atp", bufs=1) as atp, \
             tc.tile_pool(name="wst2", bufs=2) as wst2, \
             tc.tile_pool(name="gin", bufs=3) as ginp, \
             tc.tile_pool(name="osb", bufs=2) as osbp, \
             tc.tile_pool(name="ops", bufs=2, space="PSUM") as ops:
            at_sb = atp.tile([P, NH, R], BF16)
            for h in range(NH):
                nc.sync.dma_start(out=at_sb[:, h, :], in_=aspill[h])
            for htp in range(NHTP):
                wt = wst2.tile([P, KT * 256], BF16, name="wt2")
                nc.sync.dma_start(out=wt, in_=woTb[htp])
                for h2 in range(2):
                    o = htp * 2 + h2
                    g_in = ginp.tile([P, R], BF16, name="g_in")
                    nc.sync.dma_start(out=g_in, in_=gspill[o])
                    o_sb = osbp.tile([P, R], F32, name="o_sb")
                    for rh in range(LH):
                        opsum = ops.tile([P, 1024], F32, name="opsum")
                        for kt in range(KT):
                            for lq in range(2):
                                sl = slice(rh * 1024 + lq * 512,
                                           rh * 1024 + (lq + 1) * 512)
                                nc.tensor.matmul(
                                    opsum[:, lq * 512:(lq + 1) * 512],
                                    wt[:, kt * 256 + h2 * P:
                                       kt * 256 + (h2 + 1) * P],
                                    at_sb[:, kt, sl],
                                    start=(kt == 0), stop=(kt == KT - 1))
                        osl = slice(rh * 1024, (rh + 1) * 1024)
                        nc.vector.tensor_mul(
                            o_sb[:, osl], opsum, g_in[:, osl])
                    nc.sync.dma_start(out=outT[o * P:(o + 1) * P, :], in_=o_sb)

    nc.compile()
    return nc


# ===================== host side =====================

def _bf16(a):
    import ml_dtypes
    return np.ascontiguousarray(a.astype(ml_dtypes.bfloat16))


def _pack_w_256(w_io):
    # w_io [in=H, out=H] -> [NHTP, 128, KT*256]; [htp, p, kt*256+c] =
    # w_io[kt*128+p, htp*256+c]
    return _bf16(np.ascontiguousarray(
        w_io.reshape(_KT, P, _NHTP, 256).transpose(2, 1, 0, 3)
        .reshape(_NHTP, P, _KT * 256)))


def _pack_w_512(w_io):
    # w_io [in=H, out=H] -> [4, 128, KT*512]
    return _bf16(np.ascontiguousarray(
        w_io.reshape(_KT, P, 4, 512).transpose(2, 1, 0, 3)
        .reshape(4, P, _KT * 512)))


_nc_cache = [None]


def kernel(hidden_states, memory_tokens, memory_mask, norm_w,
           wq, wk, wv, wo, wg):
    import concourse.bacc as bacc

    hs = np.asarray(hidden_states, dtype=np.float32)
    mem = np.asarray(memory_tokens, dtype=np.float32)
    mask = np.asarray(memory_mask)
    norm_w = np.asarray(norm_w, dtype=np.float32)

    wq_n = (np.asarray(wq, dtype=np.float32) * norm_w[None, :]).T
    wg_n = (np.asarray(wg, dtype=np.float32) * norm_w[None, :]).T
    shared = {
        "wqTb": _pack_w_256(np.ascontiguousarray(wq_n)),
        "wgTb": _pack_w_256(np.ascontiguousarray(wg_n)),
        "woTb": _pack_w_256(np.ascontiguousarray(
            np.asarray(wo, dtype=np.float32).T)),
        "wkTb": _pack_w_512(np.ascontiguousarray(
            np.asarray(wk, dtype=np.float32).T)),
        "wvTb": _pack_w_512(np.ascontiguousarray(
            np.asarray(wv, dtype=np.float32).T)),
    }

    in_maps = []
    for c in range(_NCORES):
        b, half = c // 2, c % 2
        inp = dict(shared)
        hs_slice = hs[b, half * _R:(half + 1) * _R, :]
        inp["xT"] = np.ascontiguousarray(hs_slice.T.astype(np.float32))
        # memTb: mem[b] [M, H] -> [128, KT*256]; [p, kt*256+m] = mem[m, kt*128+p]
        inp["memTb"] = _bf16(np.ascontiguousarray(
            mem[b].T.reshape(_KT, P, _M).transpose(1, 0, 2)
            .reshape(P, _KT * _M)))
        mb = np.where(mask[b], 0.0, -50.0).astype(np.float32)
        inp["maskb"] = np.ascontiguousarray(mb.reshape(_MT, P).T)
        in_maps.append(inp)

    if _nc_cache[0] is None:
        nc = bacc.Bacc(None, target_bir_lowering=False, debug=False)
        build(nc)
        _nc_cache[0] = nc
    nc = _nc_cache[0]

    import os
    trace = os.environ.get("KERNEL_TRACE") == "1"
    res = run_bass_kernel_spmd(nc, in_maps, core_ids=list(range(_NCORES)),
                               trace=trace)
    kernel.last_result = res

    out = np.empty((_B, _L, _H), dtype=np.float32)
    for c in range(_NCORES):
        b, half = c // 2, c % 2
        out[b, half * _R:(half + 1) * _R, :] = res.results[c]["outT"].T
    return out
